# revision 1
# baseline (speedup 1.0000x reference)
"""Trainium2 Bass kernel for nn_Block_2018634629560 (dense transformer block:
gemma-normed gated attention + gated delta-net), 8-core tensor-parallel.

Strategy: two SPMD launches, head-sharded tensor parallel.
  Launch 1 (attention): 2 q-heads/core, kv-head replicated per pair;
    each core emits its partial o-projection [T, D]; host reduces
    h = x + sum(partials).
  Launch 2 (delta-net): 4 v-heads (2 k-heads)/core, chunked delta rule
    (chunk=128) with on-chip Neumann solve of (I+A)^-1; each core emits
    partial out-projection [T, D]; host reduces out = h + sum(partials).
All matmuls bf16 with fp32 PSUM accumulate; norms/decays in fp32.
"""
import math
import os
import numpy as np
import ml_dtypes

_KDBG_PHASES = int(os.environ.get("KDBG_PHASES", "3"))

import concourse.bass as bass
import concourse.tile as tile
from concourse import bacc, mybir
from concourse.bass import ts, ds
from concourse.bass_utils import run_bass_kernel_spmd

F32 = mybir.dt.float32
BF16 = mybir.dt.bfloat16
AF = mybir.ActivationFunctionType
ALU = mybir.AluOpType
BFNP = ml_dtypes.bfloat16

# ---- problem constants ----
D = 2048; HQ = 16; HKV = 4; HD = 128; ROT = 32; THETA = 10000.0; EPS = 1e-6
HK = 16; HV = 32; DK = 128; DV = 128; KCONV = 4
KEY_DIM = HK * DK; VAL_DIM = HV * DV; CONV_DIM = 2 * KEY_DIM + VAL_DIM
B = 1; T = 2048
NCORE = 8
P = 128
TT = T // P      # 16 token tiles
KT = D // P      # 16 contraction tiles
CH = 128         # delta chunk size
NCH = T // CH    # 16 chunks
NEUMANN_LEVELS = 6  # exact: A^(2^6)=A^64, last needed power for C=128


# ============================================================ launch 1 build
def build_attn():
    nc = bacc.Bacc("TRN2", target_bir_lowering=False, debug=False,
                   enable_asserts=False, num_devices=NCORE)
    dt = nc.dram_tensor
    xT = dt("xT", [D, T], BF16, kind="ExternalInput").ap()
    xr = dt("xr", [T, D], BF16, kind="ExternalInput").ap()
    wqg = dt("wqg", [D, 512], BF16, kind="ExternalInput").ap()
    wkv = dt("wkv", [D, 256], BF16, kind="ExternalInput").ap()
    wo = dt("wo", [256, D], BF16, kind="ExternalInput").ap()
    csd = dt("csd", [T, 64], F32, kind="ExternalInput").ap()
    qk1 = dt("qk1", [P, 256], BF16, kind="ExternalInput").ap()
    m4 = dt("m4", [P, 4 * 512], BF16, kind="ExternalInput").ap()
    idm = dt("idm", [P, P], BF16, kind="ExternalInput").ap()
    p1 = dt("p1", [T, D], F32, kind="ExternalOutput").ap()

    with tile.TileContext(nc) as tc:
        with tc.tile_pool(name="res", bufs=1) as res:
            # resident SBUF tensors
            xT_sb = res.tile([P, KT, T], BF16)
            wqg_sb = res.tile([P, KT, 512], BF16)
            wkv_sb = res.tile([P, KT, 256], BF16)
            cs_sb = res.tile([P, TT, 64], F32)
            qk1_sb = res.tile([P, 256], BF16)
            m4_sb = res.tile([P, 4 * 512], BF16)
            id_sb = res.tile([P, P], BF16)
            qT_sb = res.tile([P, 2, T], BF16)
            graw_sb = res.tile([P, TT, 256], BF16)
            kT_sb = res.tile([P, T], BF16)
            vE_sb = res.tile([P, TT, 132], BF16)
            gs_sb = res.tile([P, TT, 256], F32)
            ygT_sb = res.tile([P, 2, T], BF16)

            for k in range(KT):
                nc.sync.dma_start(xT_sb[:, k, :], xT[ts(k, P), :])
                nc.sync.dma_start(wqg_sb[:, k, :], wqg[ts(k, P), :])
                nc.sync.dma_start(wkv_sb[:, k, :], wkv[ts(k, P), :])
            for i in range(TT):
                nc.sync.dma_start(cs_sb[:, i, :], csd[ts(i, P), :])
            nc.sync.dma_start(qk1_sb[:], qk1[:])
            nc.sync.dma_start(m4_sb[:], m4[:])
            nc.sync.dma_start(id_sb[:], idm[:])
            nc.vector.memset(vE_sb[:, :, 128:132], 0.0)
            nc.vector.memset(vE_sb[:, :, 128:129], 1.0)
            epsD_sb = res.tile([P, 1], F32)
            nc.vector.memset(epsD_sb[:], D * EPS)
            eps_sb = res.tile([P, 1], F32)
            nc.vector.memset(eps_sb[:], EPS)

            # ---------------- phase 1: projections + norms + rope ----------
            with tc.tile_pool(name="ph1", bufs=3) as ph1, \
                 tc.tile_pool(name="ph1s", bufs=8) as ph1s, \
                 tc.tile_pool(name="psqg", bufs=2, space="PSUM") as psqg, \
                 tc.tile_pool(name="pskv", bufs=2, space="PSUM") as pskv, \
                 tc.tile_pool(name="ptr", bufs=2, space="PSUM") as ptr:
                for i in range(TT):
                    xr_t = ph1.tile([P, D], BF16, tag="xr")
                    nc.sync.dma_start(xr_t[:], xr[ts(i, P), :])
                    sqd = ph1.tile([P, D], F32, tag="sqd")
                    ssq = ph1s.tile([P, 1], F32, tag="ssq")
                    nc.vector.scalar_tensor_tensor(sqd[:], xr_t[:], 1.0,
                                                   xr_t[:], ALU.mult, ALU.mult,
                                                   accum_out=ssq[:])
                    # scale1 = sqrt(D) / sqrt(ssq + D*eps)
                    sr = ph1s.tile([P, 1], F32, tag="sr")
                    nc.scalar.activation(sr[:], ssq[:], AF.Sqrt,
                                         scale=1.0 / D, bias=eps_sb[:])
                    rr = ph1s.tile([P, 1], F32, tag="rr")
                    nc.vector.reciprocal(rr[:], sr[:])
                    scale1 = rr

                    pqg = psqg.tile([P, 512], F32)
                    pkv = pskv.tile([P, 256], F32)
                    for k in range(KT):
                        lhsT = xT_sb[:, k, ts(i, P)]
                        nc.tensor.matmul(pqg[:], lhsT, wqg_sb[:, k, :],
                                         start=(k == 0), stop=(k == KT - 1))
                        nc.tensor.matmul(pkv[:], lhsT, wkv_sb[:, k, :],
                                         start=(k == 0), stop=(k == KT - 1))
                    # per-head gemma norms (scale1 cancels for q/k)
                    for hh, (src, qkcol) in enumerate(
                            [(pqg[:, 0:128], 0), (pqg[:, 128:256], 0),
                             (pkv[:, 0:128], 128)]):
                        sq2 = ph1.tile([P, 128], F32, tag="sq2")
                        ss2 = ph1s.tile([P, 1], F32, tag="ss2")
                        nc.scalar.activation(sq2[:], src, AF.Square,
                                             accum_out=ss2[:])
                        s2 = ph1s.tile([P, 1], F32, tag="s2")
                        nc.scalar.activation(s2[:], ss2[:], AF.Sqrt,
                                             scale=1.0 / HD, bias=eps_sb[:])
                        rn = ph1s.tile([P, 1], F32, tag="rn")
                        nc.vector.reciprocal(rn[:], s2[:])
                        qn = ph1.tile([P, 128], F32, tag="qn")
                        nc.vector.scalar_tensor_tensor(
                            qn[:], src, rn[:], qk1_sb[:, qkcol:qkcol + 128],
                            ALU.mult, ALU.mult)
                        # rope on first 32 dims
                        cos = cs_sb[:, i, 0:16]; sin = cs_sb[:, i, 32:48]
                        x1 = ph1s.tile([P, 16], F32, tag="x1")
                        x2 = ph1s.tile([P, 16], F32, tag="x2")
                        nc.vector.tensor_copy(x1[:], qn[:, 0:16])
                        nc.vector.tensor_copy(x2[:], qn[:, 16:32])
                        t1 = ph1s.tile([P, 16], F32, tag="t1")
                        t2 = ph1s.tile([P, 16], F32, tag="t2")
                        nc.vector.tensor_mul(t1[:], x1[:], cos)
                        nc.vector.tensor_mul(t2[:], x2[:], sin)
                        nc.vector.tensor_sub(qn[:, 0:16], t1[:], t2[:])
                        nc.vector.tensor_mul(t1[:], x2[:], cos)
                        nc.vector.tensor_mul(t2[:], x1[:], sin)
                        nc.vector.tensor_add(qn[:, 16:32], t1[:], t2[:])
                        # cast + transpose to [hd, t]
                        qnb = ph1.tile([P, 128], BF16, tag="qnb")
                        nc.vector.tensor_copy(qnb[:], qn[:])
                        ptt = ptr.tile([P, P], BF16)
                        nc.tensor.transpose(ptt[:], qnb[:], id_sb[:])
                        dst = (qT_sb[:, hh, ts(i, P)] if hh < 2
                               else kT_sb[:, ts(i, P)])
                        nc.scalar.activation(dst, ptt[:], AF.Copy)
                    # v (needs scale1) and gate
                    nc.vector.tensor_scalar(
                        vE_sb[:, i, 0:128], pkv[:, 128:256], scale1[:], None,
                        ALU.mult)
                    nc.scalar.activation(graw_sb[:, i, :], pqg[:, 256:512],
                                         AF.Copy, scale=scale1[:])

            tc.strict_bb_all_engine_barrier()
            # gate sigmoid via exp (exp act table from here on)
            with tc.tile_pool(name="sg", bufs=3) as sgp:
              for i in range(TT):
                ge = sgp.tile([P, 256], F32, tag="ge")
                nc.scalar.activation(ge[:], graw_sb[:, i, :], AF.Exp,
                                     scale=-1.0)
                ge1 = sgp.tile([P, 256], F32, tag="ge1")
                nc.vector.tensor_scalar_add(ge1[:], ge[:], 1.0)
                nc.vector.reciprocal(gs_sb[:, i, :], ge1[:])

            # ---------------- phase 2: attention core ----------------------
            with tc.tile_pool(name="expp", bufs=20) as expp, \
                 tc.tile_pool(name="ph2", bufs=4) as ph2, \
                 tc.tile_pool(name="ph2s", bufs=4) as ph2s, \
                 tc.tile_pool(name="psT", bufs=2, space="PSUM") as psT, \
                 tc.tile_pool(name="psy", bufs=2, space="PSUM") as psy, \
                 tc.tile_pool(name="ptr2", bufs=2, space="PSUM") as ptr2:
                for h in range(2 if _KDBG_PHASES >= 2 else 0):
                    for J in range(4):
                        expTs = []
                        for i2 in range(4 * J + 4):
                            pT = psT.tile([P, 512], F32)
                            nc.tensor.matmul(
                                pT[:], kT_sb[:, ts(i2, P)],
                                qT_sb[:, h, ts(J, 512)],
                                start=True, stop=True)
                            et = expp.tile([P, 512], BF16, tag="expT")
                            nc.scalar.activation(et[:], pT[:], AF.Exp,
                                                 scale=1.0 / math.sqrt(HD))
                            r = i2 - 4 * J
                            if r >= 0:
                                nc.vector.tensor_mul(
                                    et[:], et[:], m4_sb[:, ts(r, 512)])
                            expTs.append(et)
                        for m in range(4 * J, 4 * J + 4):
                            py = psy.tile([P, 132], F32)
                            for i2 in range(m + 1):
                                nc.tensor.matmul(
                                    py[:, 0:129],
                                    expTs[i2][:, ts(m - 4 * J, P)],
                                    vE_sb[:, i2, 0:129],
                                    start=(i2 == 0), stop=(i2 == m))
                            rd = ph2s.tile([P, 1], F32, tag="rd")
                            nc.vector.reciprocal(rd[:], py[:, 128:129])
                            yg = ph2.tile([P, P], BF16, tag="yg")
                            nc.vector.scalar_tensor_tensor(
                                yg[:], py[:, 0:128], rd[:],
                                gs_sb[:, m, ts(h, P)], ALU.mult, ALU.mult)
                            pt2 = ptr2.tile([P, P], BF16)
                            nc.tensor.transpose(pt2[:], yg[:], id_sb[:])
                            nc.scalar.activation(ygT_sb[:, h, ts(m, P)],
                                                 pt2[:], AF.Copy)

            # ---------------- phase 3: o-projection ------------------------
            with tc.tile_pool(name="wo_p", bufs=1) as wo_p, \
                 tc.tile_pool(name="ph3", bufs=3) as ph3, \
                 tc.tile_pool(name="pso", bufs=4, space="PSUM") as pso:
                wo_sb = wo_p.tile([P, 2, D], BF16)
                nc.sync.dma_start(wo_sb[:, 0, :], wo[0:128, :])
                nc.sync.dma_start(wo_sb[:, 1, :], wo[128:256, :])
                for m in range(TT if _KDBG_PHASES >= 3 else 0):
                    for n in range(4):
                        po = pso.tile([P, 512], F32)
                        for h in range(2):
                            nc.tensor.matmul(po[:], ygT_sb[:, h, ts(m, P)],
                                             wo_sb[:, h, ts(n, 512)],
                                             start=(h == 0), stop=(h == 1))
                        ot = ph3.tile([P, 512], F32, tag="ot")
                        nc.vector.tensor_copy(ot[:], po[:])
                        nc.sync.dma_start(p1[ts(m, P), ts(n, 512)], ot[:])
    nc.compile()
    return nc


# ============================================================ launch 2 build
NEUM = 1  # Neumann levels beyond (I-A): applies A^2


def build_delta():
    nc = bacc.Bacc("TRN2", target_bir_lowering=False, debug=False,
                   enable_asserts=False, num_devices=NCORE)
    dt = nc.dram_tensor
    hT = dt("hT", [D, T], BF16, kind="ExternalInput").ap()
    hr = dt("hr", [T, D], BF16, kind="ExternalInput").ap()
    wqkv = dt("wqkv", [D, 1024], BF16, kind="ExternalInput").ap()
    cwt = dt("cwt", [P, 8 * KCONV], F32, kind="ExternalInput").ap()
    wz = dt("wz", [D, 512], BF16, kind="ExternalInput").ap()
    wab = dt("wab", [D, 8], BF16, kind="ExternalInput").ap()
    wout = dt("wout", [512, D], BF16, kind="ExternalInput").ap()
    dtb = dt("dtb", [P, 4], F32, kind="ExternalInput").ap()
    nega = dt("nega", [P, 4], F32, kind="ExternalInput").ap()
    nwbc = dt("nwbc", [P, 512], BF16, kind="ExternalInput").ap()
    triu = dt("triu", [P, P], F32, kind="ExternalInput").ap()
    msl = dt("msl", [P, P], BF16, kind="ExternalInput").ap()
    mli = dt("mli", [P, P], BF16, kind="ExternalInput").ap()
    idb = dt("idb", [P, P], BF16, kind="ExternalInput").ap()
    idf = dt("idf", [P, P], F32, kind="ExternalInput").ap()
    p2 = dt("p2", [T, D], F32, kind="ExternalOutput").ap()

    with tile.TileContext(nc) as tc:
      with tc.tile_pool(name="res", bufs=1) as res:
        # whole-kernel residents
        qkv_sb = res.tile([P, 8, T], BF16)      # conv+silu outputs [f, t]
        zs_sb = res.tile([P, TT, 512], BF16)    # silu(z)*nw [t, f]
        ogT_sb = res.tile([P, 4, T], BF16)      # gated o, transposed [dv, h, t]
        S_sb = res.tile([P, 4, DV], F32)        # delta state per head
        S_bf = res.tile([P, 4, DV], BF16)       # bf16 copy for matmuls
        g_sb = res.tile([P, TT, 4], F32)
        beta_sb = res.tile([P, TT, 4], F32)
        nbeta_sb = res.tile([P, TT, 4], F32)
        scale2_sb = res.tile([P, TT], F32)
        rk_all = res.tile([P, NCH, 2], F32)
        rq_all = res.tile([P, NCH, 2], F32)
        cw_sb = res.tile([P, 8, KCONV], F32)
        dtb_sb = res.tile([P, 4], F32)
        nega_sb = res.tile([P, 4], F32)
        nw_sb = res.tile([P, 512], BF16)
        triu_sb = res.tile([P, P], F32)
        msl_sb = res.tile([P, P], BF16)
        mli_sb = res.tile([P, P], BF16)
        idb_sb = res.tile([P, P], BF16)
        idf_sb = res.tile([P, P], F32)
        ones1_sb = res.tile([1, P], F32)
        onescol_sb = res.tile([P, 1], BF16)
        epsD_sb = res.tile([P, 1], F32)
        eps_sb = res.tile([P, 1], F32)
        nc.vector.memset(S_sb[:], 0.0)
        nc.vector.memset(S_bf[:], 0.0)
        nc.vector.memset(ones1_sb[:], 1.0)
        nc.vector.memset(onescol_sb[:], 1.0)
        nc.vector.memset(epsD_sb[:], D * EPS)
        nc.vector.memset(eps_sb[:], EPS)
        nc.sync.dma_start(cw_sb[:], cwt[:])
        nc.sync.dma_start(dtb_sb[:], dtb[:])
        nc.sync.dma_start(nega_sb[:], nega[:])
        nc.sync.dma_start(nw_sb[:], nwbc[:])
        nc.sync.dma_start(triu_sb[:], triu[:])
        nc.sync.dma_start(msl_sb[:], msl[:])
        nc.sync.dma_start(mli_sb[:], mli[:])
        nc.sync.dma_start(idb_sb[:], idb[:])
        nc.sync.dma_start(idf_sb[:], idf[:])

        # ============ phase A-D: projections, conv, z/ab, decay prep =======
        with tc.tile_pool(name="big1", bufs=1) as big1, \
             tc.tile_pool(name="hTp", bufs=2) as hTp, \
             tc.tile_pool(name="mxp", bufs=10) as mxp, \
             tc.tile_pool(name="wk1", bufs=2) as wk1, \
             tc.tile_pool(name="wk1s", bufs=4) as wk1s, \
             tc.tile_pool(name="psB", bufs=2, space="PSUM") as psB, \
             tc.tile_pool(name="psab", bufs=2, space="PSUM") as psab, \
             tc.tile_pool(name="ptrA", bufs=2, space="PSUM") as ptrA, \
             tc.tile_pool(name="pbcA", bufs=2, space="PSUM") as pbcA:
            wqkv_sb = big1.tile([P, KT, 1024], BF16)
            wz_sb = big1.tile([P, KT, 512], BF16)
            wab_sb = big1.tile([P, KT, 8], BF16)
            s2bc_sb = big1.tile([P, T], BF16)
            for k in range(KT):
                nc.sync.dma_start(wqkv_sb[:, k, :], wqkv[ts(k, P), :])
                nc.sync.dma_start(wz_sb[:, k, :], wz[ts(k, P), :])
                nc.sync.dma_start(wab_sb[:, k, :], wab[ts(k, P), :])

            # ---- A: scale2 per token tile + broadcast row ----
            for i in range(TT):
                hr_t = wk1.tile([P, D], BF16, tag="hr")
                nc.sync.dma_start(hr_t[:], hr[ts(i, P), :])
                sqd = wk1.tile([P, D], BF16, tag="sqd", bufs=1)
                ssq = wk1s.tile([P, 1], F32, tag="ssq")
                nc.vector.scalar_tensor_tensor(sqd[:], hr_t[:], 1.0, hr_t[:],
                                               ALU.mult, ALU.mult,
                                               accum_out=ssq[:])
                sr = wk1s.tile([P, 1], F32, tag="sr")
                nc.scalar.activation(sr[:], ssq[:], AF.Sqrt,
                                     scale=1.0 / D, bias=eps_sb[:])
                nc.vector.reciprocal(scale2_sb[:, i:i + 1], sr[:])
                ptA = ptrA.tile([1, P], F32, tag="ptA")
                nc.tensor.transpose(ptA[:], scale2_sb[:, i:i + 1], idf_sb[:])
                rowi = wk1s.tile([1, P], F32, tag="rowi")
                nc.scalar.activation(rowi[:], ptA[:], AF.Copy)
                pb = pbcA.tile([P, P], F32)
                nc.tensor.matmul(pb[:], ones1_sb[:], rowi[:],
                                 start=True, stop=True)
                nc.scalar.activation(s2bc_sb[:, ts(i, P)], pb[:], AF.Copy)

            tc.strict_bb_all_engine_barrier()
            # ---- B/C/D merged over 512-token superblocks ----
            prev_mx = [None] * 8
            ta2s = []
            for n4 in range(4):
                hT_n = hTp.tile([P, KT, 512], BF16, tag="hTn")
                for k in range(KT):
                    nc.sync.dma_start(hT_n[:, k, :],
                                      hT[ts(k, P), ts(n4, 512)])
                for F in range(8):
                    pm = psB.tile([P, 512], F32, tag="pm")
                    for k in range(KT):
                        nc.tensor.matmul(pm[:], wqkv_sb[:, k, ts(F, P)],
                                         hT_n[:, k, :],
                                         start=(k == 0), stop=(k == KT - 1))
                    m1 = mxp.tile([P, 515], BF16, tag="mxc")
                    nc.vector.tensor_mul(m1[:, 3:515], pm[:],
                                         s2bc_sb[:, ts(n4, 512)])
                    if n4 == 0:
                        nc.vector.memset(m1[:, 0:3], 0.0)
                    else:
                        nc.vector.tensor_copy(m1[:, 0:3],
                                              prev_mx[F][:, 512:515])
                    prev_mx[F] = m1
                    c0 = wk1.tile([P, 512], F32, tag="cc0")
                    nc.vector.tensor_scalar(c0[:], m1[:, 0:512],
                                            cw_sb[:, F, 0:1], None, ALU.mult)
                    for j in range(1, KCONV):
                        c1 = wk1.tile([P, 512], F32, tag=f"cc{j % 2}")
                        nc.vector.scalar_tensor_tensor(
                            c1[:], m1[:, j:512 + j], cw_sb[:, F, j:j + 1],
                            c0[:], ALU.mult, ALU.add)
                        c0 = c1
                    sg0 = wk1.tile([P, 512], F32, tag="sg0")
                    nc.scalar.activation(sg0[:], c0[:], AF.Sigmoid)
                    nc.vector.tensor_mul(qkv_sb[:, F, ts(n4, 512)], c0[:],
                                         sg0[:])
                # ---- D: z + ab for the 4 token tiles in this superblock ----
                for m in range(4 * n4, 4 * n4 + 4):
                    pz = psB.tile([P, 512], F32, tag="pm")
                    pab = psab.tile([P, 8], F32)
                    for k in range(KT):
                        lhsT = hT_n[:, k, ts(m - 4 * n4, P)]
                        nc.tensor.matmul(pz[:], lhsT, wz_sb[:, k, :],
                                         start=(k == 0), stop=(k == KT - 1))
                        nc.tensor.matmul(pab[:], lhsT, wab_sb[:, k, :],
                                         start=(k == 0), stop=(k == KT - 1))
                    zraw = wk1.tile([P, 512], F32, tag="zraw")
                    nc.vector.tensor_scalar(zraw[:], pz[:],
                                            scale2_sb[:, m:m + 1], None,
                                            ALU.mult)
                    zsg = wk1.tile([P, 512], F32, tag="zsg")
                    nc.scalar.activation(zsg[:], zraw[:], AF.Sigmoid)
                    zs1 = wk1.tile([P, 512], F32, tag="zs1")
                    nc.vector.tensor_mul(zs1[:], zraw[:], zsg[:])
                    nc.vector.tensor_mul(zs_sb[:, m, :], zs1[:], nw_sb[:])
                    ta = wk1s.tile([P, 4], F32, tag="ta")
                    nc.vector.tensor_scalar(ta[:], pab[:, 0:4],
                                            scale2_sb[:, m:m + 1], None,
                                            ALU.mult)
                    ta2 = wk1s.tile([P, 4], F32, tag="ta2", bufs=18)
                    nc.vector.tensor_add(ta2[:], ta[:], dtb_sb[:])
                    ta2s.append(ta2)
                    nc.scalar.activation(beta_sb[:, m, :], pab[:, 4:8],
                                         AF.Sigmoid,
                                         scale=scale2_sb[:, m:m + 1])
                    nc.vector.tensor_scalar_mul(nbeta_sb[:, m, :],
                                                beta_sb[:, m, :], -1.0)

            tc.strict_bb_all_engine_barrier()
            # softplus pass: g = nega * ln(1 + exp(ta2)); exp run, then ln run
            spes = []
            for m in range(TT):
                spe = wk1s.tile([P, 4], F32, tag="spe", bufs=18)
                nc.scalar.activation(spe[:], ta2s[m][:], AF.Exp)
                spes.append(spe)
            for m in range(TT):
                sp = wk1s.tile([P, 4], F32, tag="sp")
                nc.scalar.activation(sp[:], spes[m][:], AF.Ln, bias=1.0)
                nc.vector.tensor_mul(g_sb[:, m, :], sp[:], nega_sb[:])
            # ---- D3: rk/rq norm factors for all chunks (sqrt table) ----
            tc.strict_bb_all_engine_barrier()
            for n in range(NCH):
                for kh in range(2):
                    for j, fi in enumerate((kh, 2 + kh)):
                        sqk = wk1.tile([P, P], BF16, tag="sqk3")
                        nc.scalar.activation(sqk[:],
                                             qkv_sb[:, fi, ts(n, P)], AF.Square)
                        pss = psab.tile([P, 1], F32, tag="pab")
                        nc.tensor.matmul(pss[:], sqk[:], onescol_sb[:],
                                         start=True, stop=True)
                        sq = wk1s.tile([P, 1], F32, tag="sq3")
                        nc.scalar.activation(sq[:], pss[:], AF.Sqrt)
                        sqm = wk1s.tile([P, 1], F32, tag="sqm3")
                        nc.vector.tensor_scalar_max(sqm[:], sq[:], 1e-12)
                        dst = (rq_all[:, n, kh:kh + 1] if j == 0
                               else rk_all[:, n, kh:kh + 1])
                        nc.vector.reciprocal(dst, sqm[:])
            for n in range(NCH):
                nc.vector.tensor_scalar_mul(rq_all[:, n, :], rq_all[:, n, :],
                                            DK ** -0.5)
            tc.strict_bb_all_engine_barrier()

        # ============ phase E: chunked delta rule ==========================
        with tc.tile_pool(name="efp", bufs=1) as efp:
          o_all = efp.tile([P, NCH, 4, DV], F32)
          ssqo_all = efp.tile([P, NCH, 4], F32)
          with tc.tile_pool(name="wkE", bufs=6) as wkE, \
               tc.tile_pool(name="wkEs", bufs=10) as wkEs, \
               tc.tile_pool(name="uP", bufs=8) as uP, \
               tc.tile_pool(name="pbc", bufs=1, space="PSUM") as pbc, \
               tc.tile_pool(name="pg", bufs=1, space="PSUM") as pg, \
               tc.tile_pool(name="ptx", bufs=2, space="PSUM") as ptx, \
               tc.tile_pool(name="ptf", bufs=1, space="PSUM") as ptf, \
               tc.tile_pool(name="pw", bufs=1, space="PSUM") as pw, \
               tc.tile_pool(name="pch", bufs=2, space="PSUM") as pch:
              for n in range(NCH):
                  # ---- per-chunk shared prep ----
                  pcum = ptf.tile([P, 4], F32, tag="ptf")
                  nc.tensor.matmul(pcum[:], triu_sb[:], g_sb[:, n, :],
                                   start=True, stop=True)
                  cum_sb = wkEs.tile([P, 4], F32, tag="cum")
                  nc.scalar.activation(cum_sb[:], pcum[:], AF.Copy)
                  di_sb = wkEs.tile([P, 4], F32, tag="di")
                  nc.scalar.activation(di_sb[:], pcum[:], AF.Exp)
                  cumrows = []; betarows = []
                  for hh in range(4):
                      ptc = ptf.tile([1, P], F32, tag="ptf")
                      nc.tensor.transpose(ptc[:], cum_sb[:, hh:hh + 1],
                                          idf_sb[:])
                      cr = wkEs.tile([1, P], F32, tag="cumrow")
                      nc.scalar.activation(cr[:], ptc[:], AF.Copy)
                      cumrows.append(cr)
                      ptb = ptf.tile([1, P], F32, tag="ptf")
                      nc.tensor.transpose(ptb[:], beta_sb[:, n, hh:hh + 1],
                                          idf_sb[:])
                      br = wkEs.tile([1, P], F32, tag="betarow")
                      nc.scalar.activation(br[:], ptb[:], AF.Copy)
                      betarows.append(br)
                  rkrows = []
                  for kh in range(2):
                      ptk = ptf.tile([1, P], F32, tag="ptf")
                      nc.tensor.transpose(ptk[:], rk_all[:, n, kh:kh + 1],
                                          idf_sb[:])
                      rkr = wkEs.tile([1, P], F32, tag="rkr")
                      nc.scalar.activation(rkr[:], ptk[:], AF.Copy)
                      rkrows.append(rkr)
                  for h in range(4):
                      kh = h // 2
                      qt = qkv_sb[:, kh, ts(n, P)]
                      kt = qkv_sb[:, 2 + kh, ts(n, P)]
                      vt = qkv_sb[:, 4 + h, ts(n, P)]
                      cum_col = cum_sb[:, h:h + 1]
                      di_col = di_sb[:, h:h + 1]
                      beta_col = beta_sb[:, n, h:h + 1]
                      nbeta_col = nbeta_sb[:, n, h:h + 1]
                      rq_col = rq_all[:, n, kh:kh + 1]
                      # broadcasts: [cum | beta | rk]
                      pb = pbc.tile([P, 384], F32)
                      nc.tensor.matmul(pb[:, 0:128], ones1_sb[:],
                                       cumrows[h][:], start=True, stop=True)
                      nc.tensor.matmul(pb[:, 128:256], ones1_sb[:],
                                       betarows[h][:], start=True, stop=True)
                      nc.tensor.matmul(pb[:, 256:384], ones1_sb[:],
                                       rkrows[kh][:], start=True, stop=True)
                      gl = wkEs.tile([P, 1], F32, tag="gl")
                      nc.scalar.activation(gl[:], pb[:, 127:128], AF.Copy)
                      gend = wkEs.tile([P, 1], F32, tag="gend")
                      nc.scalar.activation(gend[:], gl[:], AF.Exp)
                      dl = wkE.tile([P, P], F32, tag="dl")
                      nc.vector.tensor_scalar(dl[:], pb[:, 0:128], cum_col, 0.0,
                                              ALU.subtract, ALU.max)
                      dm = wkE.tile([P, P], BF16, tag="dm")
                      nc.scalar.activation(dm[:], dl[:], AF.Exp, scale=-1.0)
                      am = wkE.tile([P, P], BF16, tag="am")
                      nc.vector.tensor_mul(am[:], dm[:], msl_sb[:])
                      pmm = wkE.tile([P, P], BF16, tag="pmm")
                      nc.vector.tensor_mul(pmm[:], dm[:], mli_sb[:])
                      dibc = wkE.tile([P, P], BF16, tag="dibc")
                      nc.scalar.activation(dibc[:], pb[:, 0:128], AF.Exp)
                      drbc = wkE.tile([P, P], BF16, tag="drbc")
                      nc.scalar.activation(drbc[:], pb[:, 0:128], AF.Exp,
                                           scale=-1.0, bias=gl[:])
                      kbar = wkE.tile([P, P], BF16, tag="kbar")
                      nc.vector.tensor_mul(kbar[:], kt, pb[:, 256:384])
                      qtil = wkE.tile([P, P], BF16, tag="qtil")
                      nc.vector.tensor_mul(qtil[:], qt, dibc[:])
                      ktil = wkE.tile([P, P], BF16, tag="ktil")
                      nc.vector.tensor_mul(ktil[:], kbar[:], drbc[:])
                      # grams
                      pgr = pg.tile([P, 256], F32)
                      nc.tensor.matmul(pgr[:, 0:128], kbar[:], kbar[:],
                                       start=True, stop=True)
                      nc.tensor.matmul(pgr[:, 128:256], qt, kbar[:],
                                       start=True, stop=True)
                      A_sb = wkE.tile([P, P], BF16, tag="A")
                      nc.vector.scalar_tensor_tensor(A_sb[:], pgr[:, 0:128],
                                                     beta_col, am[:],
                                                     ALU.mult, ALU.mult)
                      P_sb = wkE.tile([P, P], BF16, tag="Pm")
                      nc.vector.tensor_mul(P_sb[:], pgr[:, 128:256], pmm[:])
                      # transposes: Abar | Pbar | vT | ktilT
                      ptt = ptx.tile([P, 512], BF16, tag="ptx")
                      nc.tensor.transpose(ptt[:, 0:128], A_sb[:], idb_sb[:])
                      nc.tensor.transpose(ptt[:, 128:256], P_sb[:], idb_sb[:])
                      nc.tensor.transpose(ptt[:, 256:384], vt, idb_sb[:])
                      nc.tensor.transpose(ptt[:, 384:512], ktil[:], idb_sb[:])
                      abar = wkE.tile([P, P], BF16, tag="abar")
                      nc.scalar.activation(abar[:], ptt[:, 0:128], AF.Copy)
                      pbar = wkE.tile([P, P], BF16, tag="pbar")
                      nc.scalar.activation(pbar[:], ptt[:, 128:256], AF.Copy)
                      ktT = wkE.tile([P, P], BF16, tag="ktT")
                      nc.scalar.activation(ktT[:], ptt[:, 384:512], AF.Copy)
                      # S-dependent chain
                      Sh = S_sb[:, h, :]
                      Shb = S_bf[:, h, :]
                      pc1 = pch.tile([P, P], F32, tag="pc")
                      nc.tensor.matmul(pc1[:], qtil[:], Shb, start=True, stop=True)
                      o_tmp = wkE.tile([P, P], F32, tag="o_tmp")
                      nc.vector.tensor_scalar(o_tmp[:], pc1[:], rq_col, None,
                                              ALU.mult)
                      pc2 = pch.tile([P, P], F32, tag="pc")
                      nc.tensor.matmul(pc2[:], kbar[:], Shb, start=True, stop=True)
                      tpred = wkE.tile([P, P], F32, tag="tpred")
                      nc.vector.tensor_scalar(tpred[:], pc2[:], di_col,
                                              nbeta_col, ALU.mult, ALU.mult)
                      u0 = uP.tile([P, P], BF16, tag="u")
                      nc.vector.scalar_tensor_tensor(u0[:], ptt[:, 256:384],
                                                     beta_col, tpred[:],
                                                     ALU.mult, ALU.add)
                      pc3 = pch.tile([P, P], F32, tag="pc")
                      nc.tensor.matmul(pc3[:], abar[:], u0[:], start=True,
                                       stop=True)
                      u1 = uP.tile([P, P], BF16, tag="u")
                      nc.vector.tensor_sub(u1[:], u0[:], pc3[:])
                      ucur = u1
                      asb, absb = A_sb, abar
                      for lev in range(NEUM):
                          pw1 = pw.tile([P, P], F32, tag="pw")
                          nc.tensor.matmul(pw1[:], asb[:], absb[:], start=True,
                                           stop=True)
                          ab2 = wkE.tile([P, P], BF16, tag=f"ab2_{lev}")
                          nc.scalar.activation(ab2[:], pw1[:], AF.Copy)
                          if lev + 1 < NEUM:
                              pw2 = pw.tile([P, P], F32, tag="pw")
                              nc.tensor.matmul(pw2[:], absb[:], asb[:],
                                               start=True, stop=True)
                              a2 = wkE.tile([P, P], BF16, tag=f"a2_{lev}")
                              nc.scalar.activation(a2[:], pw2[:], AF.Copy)
                              asb = a2
                          pc4 = pch.tile([P, P], F32, tag="pc")
                          nc.tensor.matmul(pc4[:], ab2[:], ucur[:], start=True,
                                           stop=True)
                          unext = uP.tile([P, P], BF16, tag="u")
                          nc.vector.tensor_add(unext[:], ucur[:], pc4[:])
                          ucur = unext
                          absb = ab2
                      # o = rq*(qtil S0 + P u)
                      pc5 = pch.tile([P, P], F32, tag="pc")
                      nc.tensor.matmul(pc5[:], pbar[:], ucur[:], start=True,
                                       stop=True)
                      o_sb = o_all[:, n, h, :]
                      nc.vector.scalar_tensor_tensor(o_sb, pc5[:], rq_col,
                                                     o_tmp[:], ALU.mult, ALU.add)
                      sqo = wkE.tile([P, P], F32, tag="sqo")
                      nc.scalar.activation(sqo[:], o_sb, AF.Square,
                                           accum_out=ssqo_all[:, n, h:h + 1])
                      # state update: S = gend*S + ktil @ u
                      pc6 = pch.tile([P, P], F32, tag="pc")
                      nc.tensor.matmul(pc6[:], ktT[:], ucur[:], start=True,
                                       stop=True)
                      nc.vector.scalar_tensor_tensor(Sh, Sh, gend[:], pc6[:],
                                                     ALU.mult, ALU.add)
                      nc.vector.tensor_copy(Shb, Sh)

        # ============ phase F: gated rms + out projection ==================
          tc.strict_bb_all_engine_barrier()
          with tc.tile_pool(name="wo2", bufs=1) as wo2, \
             tc.tile_pool(name="wkF", bufs=4) as wkF, \
             tc.tile_pool(name="wkFs", bufs=4) as wkFs, \
             tc.tile_pool(name="ptxF", bufs=2, space="PSUM") as ptxF, \
             tc.tile_pool(name="psF", bufs=4, space="PSUM") as psF:
            for n in range(NCH):
                sro4 = wkFs.tile([P, 4], F32, tag="sro4")
                nc.scalar.activation(sro4[:], ssqo_all[:, n, :], AF.Sqrt,
                                     scale=1.0 / DV, bias=eps_sb[:])
                rro4 = wkFs.tile([P, 4], F32, tag="rro4")
                nc.vector.reciprocal(rro4[:], sro4[:])
                for h in range(4):
                    og = wkF.tile([P, P], BF16, tag="og")
                    nc.vector.scalar_tensor_tensor(og[:], o_all[:, n, h, :],
                                                   rro4[:, h:h + 1],
                                                   zs_sb[:, n, ts(h, P)],
                                                   ALU.mult, ALU.mult)
                    pto = ptxF.tile([P, P], BF16, tag="pto")
                    nc.tensor.transpose(pto[:], og[:], idb_sb[:])
                    nc.scalar.activation(ogT_sb[:, h, ts(n, P)], pto[:],
                                         AF.Copy)
            wout_sb = wo2.tile([P, 4, D], BF16)
            for h in range(4):
                nc.sync.dma_start(wout_sb[:, h, :], wout[ts(h, P), :])
            for m in range(TT):
                for nn in range(4):
                    po = psF.tile([P, 512], F32)
                    for h in range(4):
                        nc.tensor.matmul(po[:], ogT_sb[:, h, ts(m, P)],
                                         wout_sb[:, h, ts(nn, 512)],
                                         start=(h == 0), stop=(h == 3))
                    ot = wkF.tile([P, 512], F32, tag="ot")
                    nc.scalar.activation(ot[:], po[:], AF.Copy)
                    nc.sync.dma_start(p2[ts(m, P), ts(nn, 512)], ot[:])
    nc.compile()
    return nc


def _prep_delta_inputs(h, ln2_w, dn_qkv_w, dn_z_w, dn_b_w, dn_a_w, conv_w,
                       dt_bias, A_log, dn_norm_w, dn_out_w):
    ln2f = (1.0 + ln2_w.astype(np.float32))
    hT_np = _bf(h.T)
    hr_np = _bf(h)
    a2 = np.arange(P)
    triu_np = (a2[:, None] <= a2[None, :]).astype(np.float32)
    msl_np = _bf((a2[:, None] > a2[None, :]).astype(np.float32))
    mli_np = _bf((a2[:, None] >= a2[None, :]).astype(np.float32))
    idb_np = _bf(np.eye(P, dtype=np.float32))
    idf_np = np.eye(P, dtype=np.float32)
    nw_np = _bf(np.tile(dn_norm_w.astype(np.float32)[None, :], (P, 4)))
    ins = []
    for c in range(NCORE):
        khs = [2 * c, 2 * c + 1]
        vhs = [4 * c + j for j in range(4)]
        qrows = np.concatenate([dn_qkv_w[kh * DK:(kh + 1) * DK] for kh in khs])
        krows = np.concatenate([dn_qkv_w[KEY_DIM + kh * DK:
                                         KEY_DIM + (kh + 1) * DK] for kh in khs])
        vrows = dn_qkv_w[2 * KEY_DIM + vhs[0] * DV:
                         2 * KEY_DIM + (vhs[-1] + 1) * DV]
        rows = np.concatenate([qrows, krows, vrows])  # [1024, D]
        wqkv_np = _bf((rows * ln2f[None, :]).T)
        crow_q = np.concatenate([conv_w[kh * DK:(kh + 1) * DK, 0, :]
                                 for kh in khs])
        crow_k = np.concatenate([conv_w[KEY_DIM + kh * DK:
                                        KEY_DIM + (kh + 1) * DK, 0, :]
                                 for kh in khs])
        crow_v = conv_w[2 * KEY_DIM + vhs[0] * DV:
                        2 * KEY_DIM + (vhs[-1] + 1) * DV, 0, :]
        crows = np.concatenate([crow_q, crow_k, crow_v])  # [1024, 4]
        cwt_np = np.ascontiguousarray(
            crows.reshape(8, P, KCONV).transpose(1, 0, 2).reshape(P, 8 * KCONV)
        ).astype(np.float32)
        zrows = dn_z_w[vhs[0] * DV:(vhs[-1] + 1) * DV]
        wz_np = _bf((zrows * ln2f[None, :]).T)
        abrows = np.concatenate([dn_a_w[vhs[0]:vhs[-1] + 1],
                                 dn_b_w[vhs[0]:vhs[-1] + 1]])
        wab_np = _bf((abrows * ln2f[None, :]).T)
        wout_np = _bf(dn_out_w[:, vhs[0] * DV:(vhs[-1] + 1) * DV].T)
        dtb_np = np.tile(dt_bias[vhs[0]:vhs[-1] + 1][None, :],
                         (P, 1)).astype(np.float32)
        nega_np = np.tile(-np.exp(A_log[vhs[0]:vhs[-1] + 1])[None, :],
                          (P, 1)).astype(np.float32)
        ins.append(dict(hT=hT_np, hr=hr_np, wqkv=wqkv_np, cwt=cwt_np,
                        wz=wz_np, wab=wab_np, wout=wout_np, dtb=dtb_np,
                        nega=nega_np, nwbc=nw_np, triu=triu_np, msl=msl_np,
                        mli=mli_np, idb=idb_np, idf=idf_np))
    return ins


def _get_delta_nc():
    if "delta" not in _CACHE:
        _CACHE["delta"] = build_delta()
    return _CACHE["delta"]


def run_delta(h, ln2_w, dn_qkv_w, dn_z_w, dn_b_w, dn_a_w, conv_w,
              dt_bias, A_log, dn_norm_w, dn_out_w):
    nc2 = _get_delta_nc()
    ins2 = _prep_delta_inputs(h, ln2_w, dn_qkv_w, dn_z_w, dn_b_w, dn_a_w,
                              conv_w, dt_bias, A_log, dn_norm_w, dn_out_w)
    res2 = run_bass_kernel_spmd(nc2, ins2, core_ids=list(range(NCORE)))
    out = h.astype(np.float32).copy()
    for c in range(NCORE):
        out += res2.results[c]["p2"]
    return out


# ============================================================ host helpers
def _bf(a):
    return np.ascontiguousarray(a.astype(BFNP))


def _prep_attn_inputs(x, input_pos, ln1_w, q_w, k_w, v_w, o_w, qn_w, kn_w):
    x2 = x.reshape(T, D).astype(np.float32)
    ln1f = (1.0 + ln1_w.astype(np.float32))
    xT_np = _bf(x2.T)
    xr_np = _bf(x2)
    inv_freq = 1.0 / THETA ** (np.arange(0, ROT, 2, dtype=np.float32) / ROT)
    fr = input_pos.astype(np.float32)[:, None] * inv_freq[None, :]
    cos = np.cos(fr).astype(np.float32); sin = np.sin(fr).astype(np.float32)
    csd_np = np.concatenate([cos, cos, sin, sin], axis=1)
    qk1_np = _bf(np.concatenate(
        [np.tile(1.0 + qn_w[None, :], (P, 1)),
         np.tile(1.0 + kn_w[None, :], (P, 1))], axis=1))
    a = np.arange(P)[:, None]; b = np.arange(512)[None, :]
    m4_np = _bf(np.concatenate(
        [(a + 128 * r <= b).astype(np.float32) for r in range(4)], axis=1))
    idm_np = _bf(np.eye(P, dtype=np.float32))
    ins = []
    for c in range(NCORE):
        qh = [2 * c, 2 * c + 1]; kvh = c // 2
        qrows = np.concatenate([q_w[h * 256: h * 256 + 128] for h in qh]
                               + [q_w[h * 256 + 128: h * 256 + 256] for h in qh])
        wqg_np = _bf((qrows * ln1f[None, :]).T)
        kvrows = np.concatenate([k_w[kvh * 128: kvh * 128 + 128],
                                 v_w[kvh * 128: kvh * 128 + 128]])
        wkv_np = _bf((kvrows * ln1f[None, :]).T)
        wo_np = _bf(o_w[:, 2 * c * 128: 2 * c * 128 + 256].T)
        ins.append(dict(xT=xT_np, xr=xr_np, wqg=wqg_np, wkv=wkv_np,
                        wo=wo_np, csd=csd_np, qk1=qk1_np, m4=m4_np,
                        idm=idm_np))
    return ins


_CACHE = {}


def _get_attn_nc():
    if "attn" not in _CACHE:
        _CACHE["attn"] = build_attn()
    return _CACHE["attn"]


def kernel(x, input_pos, ln1_w, ln2_w, q_w, k_w, v_w, o_w, qn_w, kn_w,
           dn_qkv_w, dn_z_w, dn_b_w, dn_a_w, conv_w, dt_bias, A_log,
           dn_norm_w, dn_out_w):
    x = np.asarray(x); input_pos = np.asarray(input_pos)
    args = dict(x=x, input_pos=input_pos, ln1_w=np.asarray(ln1_w),
                ln2_w=np.asarray(ln2_w), q_w=np.asarray(q_w),
                k_w=np.asarray(k_w), v_w=np.asarray(v_w), o_w=np.asarray(o_w),
                qn_w=np.asarray(qn_w), kn_w=np.asarray(kn_w))
    nc1 = _get_attn_nc()
    ins1 = _prep_attn_inputs(x, input_pos, args["ln1_w"], args["q_w"],
                             args["k_w"], args["v_w"], args["o_w"],
                             args["qn_w"], args["kn_w"])
    res1 = run_bass_kernel_spmd(nc1, ins1, core_ids=list(range(NCORE)))
    h = x.reshape(T, D).astype(np.float32).copy()
    for c in range(NCORE):
        h += res1.results[c]["p1"]

    out = run_delta(h, np.asarray(ln2_w), np.asarray(dn_qkv_w),
                    np.asarray(dn_z_w), np.asarray(dn_b_w),
                    np.asarray(dn_a_w), np.asarray(conv_w),
                    np.asarray(dt_bias), np.asarray(A_log),
                    np.asarray(dn_norm_w), np.asarray(dn_out_w))
    return out.reshape(B, T, D).astype(np.float32)



# revision 13
# speedup vs baseline: 1.2200x; 1.2200x over previous
"""Trainium2 Bass kernel for nn_Block_2018634629560 (dense transformer block:
gemma-normed gated attention + gated delta-net), 8-core tensor-parallel.

Strategy: two SPMD launches, head-sharded tensor parallel.
  Launch 1 (attention): 2 q-heads/core, kv-head replicated per pair;
    each core emits its partial o-projection [T, D]; host reduces
    h = x + sum(partials).
  Launch 2 (delta-net): 4 v-heads (2 k-heads)/core, chunked delta rule
    (chunk=128) with on-chip Neumann solve of (I+A)^-1; each core emits
    partial out-projection [T, D]; host reduces out = h + sum(partials).
All matmuls bf16 with fp32 PSUM accumulate; norms/decays in fp32.
"""
import math
import os
import numpy as np
import ml_dtypes

_KDBG_PHASES = int(os.environ.get("KDBG_PHASES", "3"))

import concourse.bass as bass
import concourse.tile as tile
from concourse import bacc, mybir
from concourse.bass import ts, ds
from concourse.bass_utils import run_bass_kernel_spmd

F32 = mybir.dt.float32
BF16 = mybir.dt.bfloat16
AF = mybir.ActivationFunctionType
ALU = mybir.AluOpType
BFNP = ml_dtypes.bfloat16

# ---- problem constants ----
D = 2048; HQ = 16; HKV = 4; HD = 128; ROT = 32; THETA = 10000.0; EPS = 1e-6
HK = 16; HV = 32; DK = 128; DV = 128; KCONV = 4
KEY_DIM = HK * DK; VAL_DIM = HV * DV; CONV_DIM = 2 * KEY_DIM + VAL_DIM
B = 1; T = 2048
NCORE = 8
P = 128
TT = T // P      # 16 token tiles
KT = D // P      # 16 contraction tiles
CH = 128         # delta chunk size
NCH = T // CH    # 16 chunks
NEUMANN_LEVELS = 6  # exact: A^(2^6)=A^64, last needed power for C=128


# ============================================================ launch 1 build
def build_attn():
    nc = bacc.Bacc("TRN2", target_bir_lowering=False, debug=False,
                   enable_asserts=False, num_devices=NCORE)
    dt = nc.dram_tensor
    xT = dt("xT", [D, T], BF16, kind="ExternalInput").ap()
    xr = dt("xr", [T, D], BF16, kind="ExternalInput").ap()
    wqg = dt("wqg", [D, 512], BF16, kind="ExternalInput").ap()
    wkv = dt("wkv", [D, 256], BF16, kind="ExternalInput").ap()
    wo = dt("wo", [256, D], BF16, kind="ExternalInput").ap()
    csd = dt("csd", [T, 64], F32, kind="ExternalInput").ap()
    qk1 = dt("qk1", [P, 256], BF16, kind="ExternalInput").ap()
    m4 = dt("m4", [P, 4 * 512], BF16, kind="ExternalInput").ap()
    idm = dt("idm", [P, P], BF16, kind="ExternalInput").ap()
    p1 = dt("p1", [T, D], F32, kind="ExternalOutput").ap()

    with tile.TileContext(nc) as tc:
        with tc.tile_pool(name="res", bufs=1) as res:
            # resident SBUF tensors
            xT_sb = res.tile([P, KT, T], BF16)
            wqg_sb = res.tile([P, KT, 512], BF16)
            wkv_sb = res.tile([P, KT, 256], BF16)
            cs_sb = res.tile([P, TT, 64], F32)
            qk1_sb = res.tile([P, 256], BF16)
            m4_sb = res.tile([P, 4 * 512], BF16)
            id_sb = res.tile([P, P], BF16)
            qT_sb = res.tile([P, 2, T], BF16)
            graw_sb = res.tile([P, TT, 256], BF16)
            kT_sb = res.tile([P, T], BF16)
            vE_sb = res.tile([P, TT, 132], BF16)
            gs_sb = res.tile([P, TT, 256], F32)
            ygT_sb = res.tile([P, 2, T], BF16)

            for k in range(KT):
                nc.sync.dma_start(xT_sb[:, k, :], xT[ts(k, P), :])
                nc.sync.dma_start(wqg_sb[:, k, :], wqg[ts(k, P), :])
                nc.sync.dma_start(wkv_sb[:, k, :], wkv[ts(k, P), :])
            for i in range(TT):
                nc.sync.dma_start(cs_sb[:, i, :], csd[ts(i, P), :])
            nc.sync.dma_start(qk1_sb[:], qk1[:])
            nc.sync.dma_start(m4_sb[:], m4[:])
            nc.sync.dma_start(id_sb[:], idm[:])
            nc.vector.memset(vE_sb[:, :, 128:132], 0.0)
            nc.vector.memset(vE_sb[:, :, 128:129], 1.0)
            epsD_sb = res.tile([P, 1], F32)
            nc.vector.memset(epsD_sb[:], D * EPS)
            eps_sb = res.tile([P, 1], F32)
            nc.vector.memset(eps_sb[:], EPS)

            # ---------------- phase 1: projections + norms + rope ----------
            with tc.tile_pool(name="ph1", bufs=3) as ph1, \
                 tc.tile_pool(name="ph1s", bufs=8) as ph1s, \
                 tc.tile_pool(name="psqg", bufs=2, space="PSUM") as psqg, \
                 tc.tile_pool(name="pskv", bufs=2, space="PSUM") as pskv, \
                 tc.tile_pool(name="ptr", bufs=2, space="PSUM") as ptr:
                for i in range(TT):
                    xr_t = ph1.tile([P, D], BF16, tag="xr")
                    nc.sync.dma_start(xr_t[:], xr[ts(i, P), :])
                    sqd = ph1.tile([P, D], F32, tag="sqd")
                    ssq = ph1s.tile([P, 1], F32, tag="ssq")
                    nc.vector.scalar_tensor_tensor(sqd[:], xr_t[:], 1.0,
                                                   xr_t[:], ALU.mult, ALU.mult,
                                                   accum_out=ssq[:])
                    # scale1 = sqrt(D) / sqrt(ssq + D*eps)
                    sr = ph1s.tile([P, 1], F32, tag="sr")
                    nc.scalar.activation(sr[:], ssq[:], AF.Sqrt,
                                         scale=1.0 / D, bias=eps_sb[:])
                    rr = ph1s.tile([P, 1], F32, tag="rr")
                    nc.vector.reciprocal(rr[:], sr[:])
                    scale1 = rr

                    pqg = psqg.tile([P, 512], F32)
                    pkv = pskv.tile([P, 256], F32)
                    for k in range(KT):
                        lhsT = xT_sb[:, k, ts(i, P)]
                        nc.tensor.matmul(pqg[:], lhsT, wqg_sb[:, k, :],
                                         start=(k == 0), stop=(k == KT - 1))
                        nc.tensor.matmul(pkv[:], lhsT, wkv_sb[:, k, :],
                                         start=(k == 0), stop=(k == KT - 1))
                    # per-head gemma norms (scale1 cancels for q/k)
                    for hh, (src, qkcol) in enumerate(
                            [(pqg[:, 0:128], 0), (pqg[:, 128:256], 0),
                             (pkv[:, 0:128], 128)]):
                        sq2 = ph1.tile([P, 128], F32, tag="sq2")
                        ss2 = ph1s.tile([P, 1], F32, tag="ss2")
                        nc.scalar.activation(sq2[:], src, AF.Square,
                                             accum_out=ss2[:])
                        s2 = ph1s.tile([P, 1], F32, tag="s2")
                        nc.scalar.activation(s2[:], ss2[:], AF.Sqrt,
                                             scale=1.0 / HD, bias=eps_sb[:])
                        rn = ph1s.tile([P, 1], F32, tag="rn")
                        nc.vector.reciprocal(rn[:], s2[:])
                        qn = ph1.tile([P, 128], F32, tag="qn")
                        nc.vector.scalar_tensor_tensor(
                            qn[:], src, rn[:], qk1_sb[:, qkcol:qkcol + 128],
                            ALU.mult, ALU.mult)
                        # rope on first 32 dims
                        cos = cs_sb[:, i, 0:16]; sin = cs_sb[:, i, 32:48]
                        x1 = ph1s.tile([P, 16], F32, tag="x1")
                        x2 = ph1s.tile([P, 16], F32, tag="x2")
                        nc.vector.tensor_copy(x1[:], qn[:, 0:16])
                        nc.vector.tensor_copy(x2[:], qn[:, 16:32])
                        t1 = ph1s.tile([P, 16], F32, tag="t1")
                        t2 = ph1s.tile([P, 16], F32, tag="t2")
                        nc.vector.tensor_mul(t1[:], x1[:], cos)
                        nc.vector.tensor_mul(t2[:], x2[:], sin)
                        nc.vector.tensor_sub(qn[:, 0:16], t1[:], t2[:])
                        nc.vector.tensor_mul(t1[:], x2[:], cos)
                        nc.vector.tensor_mul(t2[:], x1[:], sin)
                        nc.vector.tensor_add(qn[:, 16:32], t1[:], t2[:])
                        # cast + transpose to [hd, t]
                        qnb = ph1.tile([P, 128], BF16, tag="qnb")
                        nc.vector.tensor_copy(qnb[:], qn[:])
                        ptt = ptr.tile([P, P], BF16)
                        nc.tensor.transpose(ptt[:], qnb[:], id_sb[:])
                        dst = (qT_sb[:, hh, ts(i, P)] if hh < 2
                               else kT_sb[:, ts(i, P)])
                        nc.scalar.activation(dst, ptt[:], AF.Copy)
                    # v (needs scale1) and gate
                    nc.vector.tensor_scalar(
                        vE_sb[:, i, 0:128], pkv[:, 128:256], scale1[:], None,
                        ALU.mult)
                    nc.scalar.activation(graw_sb[:, i, :], pqg[:, 256:512],
                                         AF.Copy, scale=scale1[:])

            tc.strict_bb_all_engine_barrier()
            # gate sigmoid via exp (exp act table from here on)
            with tc.tile_pool(name="sg", bufs=3) as sgp:
              for i in range(TT):
                ge = sgp.tile([P, 256], F32, tag="ge")
                nc.scalar.activation(ge[:], graw_sb[:, i, :], AF.Exp,
                                     scale=-1.0)
                ge1 = sgp.tile([P, 256], F32, tag="ge1")
                nc.vector.tensor_scalar_add(ge1[:], ge[:], 1.0)
                nc.vector.reciprocal(gs_sb[:, i, :], ge1[:])

            # ---------------- phase 2: attention core ----------------------
            with tc.tile_pool(name="expp", bufs=20) as expp, \
                 tc.tile_pool(name="ph2", bufs=4) as ph2, \
                 tc.tile_pool(name="ph2s", bufs=4) as ph2s, \
                 tc.tile_pool(name="psT", bufs=2, space="PSUM") as psT, \
                 tc.tile_pool(name="psy", bufs=2, space="PSUM") as psy, \
                 tc.tile_pool(name="ptr2", bufs=2, space="PSUM") as ptr2:
                for h in range(2 if _KDBG_PHASES >= 2 else 0):
                    for J in range(4):
                        expTs = []
                        for i2 in range(4 * J + 4):
                            pT = psT.tile([P, 512], F32)
                            nc.tensor.matmul(
                                pT[:], kT_sb[:, ts(i2, P)],
                                qT_sb[:, h, ts(J, 512)],
                                start=True, stop=True)
                            et = expp.tile([P, 512], BF16, tag="expT")
                            nc.scalar.activation(et[:], pT[:], AF.Exp,
                                                 scale=1.0 / math.sqrt(HD))
                            r = i2 - 4 * J
                            if r >= 0:
                                nc.vector.tensor_mul(
                                    et[:], et[:], m4_sb[:, ts(r, 512)])
                            expTs.append(et)
                        for m in range(4 * J, 4 * J + 4):
                            py = psy.tile([P, 132], F32)
                            for i2 in range(m + 1):
                                nc.tensor.matmul(
                                    py[:, 0:129],
                                    expTs[i2][:, ts(m - 4 * J, P)],
                                    vE_sb[:, i2, 0:129],
                                    start=(i2 == 0), stop=(i2 == m))
                            rd = ph2s.tile([P, 1], F32, tag="rd")
                            nc.vector.reciprocal(rd[:], py[:, 128:129])
                            yg = ph2.tile([P, P], BF16, tag="yg")
                            nc.vector.scalar_tensor_tensor(
                                yg[:], py[:, 0:128], rd[:],
                                gs_sb[:, m, ts(h, P)], ALU.mult, ALU.mult)
                            pt2 = ptr2.tile([P, P], BF16)
                            nc.tensor.transpose(pt2[:], yg[:], id_sb[:])
                            nc.scalar.activation(ygT_sb[:, h, ts(m, P)],
                                                 pt2[:], AF.Copy)

            # ---------------- phase 3: o-projection ------------------------
            with tc.tile_pool(name="wo_p", bufs=1) as wo_p, \
                 tc.tile_pool(name="ph3", bufs=3) as ph3, \
                 tc.tile_pool(name="pso", bufs=4, space="PSUM") as pso:
                wo_sb = wo_p.tile([P, 2, D], BF16)
                nc.sync.dma_start(wo_sb[:, 0, :], wo[0:128, :])
                nc.sync.dma_start(wo_sb[:, 1, :], wo[128:256, :])
                for m in range(TT if _KDBG_PHASES >= 3 else 0):
                    for n in range(4):
                        po = pso.tile([P, 512], F32)
                        for h in range(2):
                            nc.tensor.matmul(po[:], ygT_sb[:, h, ts(m, P)],
                                             wo_sb[:, h, ts(n, 512)],
                                             start=(h == 0), stop=(h == 1))
                        ot = ph3.tile([P, 512], F32, tag="ot")
                        nc.vector.tensor_copy(ot[:], po[:])
                        nc.sync.dma_start(p1[ts(m, P), ts(n, 512)], ot[:])
    nc.compile()
    return nc



def _load_act_set(nc, set_id):
    ld = mybir.InstLoadActFuncSet(
        name=nc.get_next_instruction_name(), ins=[], outs=[],
        act_func_set_id=set_id)
    ld.engine = mybir.EngineType.Activation
    nc.scalar.add_instruction(ld)

# ============================================================ launch 2 build
MASKV = -32768.0   # additive mask; exp(x + MASKV) == 0 exactly in f32
LNEPS = 1e-24      # bias inside ln() to avoid ln(0)


def build_delta():
    nc = bacc.Bacc("TRN2", target_bir_lowering=False, debug=False,
                   enable_asserts=False, num_devices=NCORE)
    dt = nc.dram_tensor
    hT = dt("hT", [D, T], BF16, kind="ExternalInput").ap()
    wqkv = dt("wqkv", [D, 1024], BF16, kind="ExternalInput").ap()
    cwt = dt("cwt", [P, 8 * KCONV], F32, kind="ExternalInput").ap()
    wz = dt("wz", [D, 512], BF16, kind="ExternalInput").ap()
    wout = dt("wout", [512, D], BF16, kind="ExternalInput").ap()
    s2c = dt("s2c", [P, TT], F32, kind="ExternalInput").ap()
    s2bc = dt("s2bc", [P, T], BF16, kind="ExternalInput").ap()
    # host-precomputed per-chunk gate columns: [P, NCH, 4] f32 each
    cumc = dt("cumc", [P, NCH, 4], F32, kind="ExternalInput").ap()
    betac = dt("betac", [P, NCH, 4], F32, kind="ExternalInput").ap()
    negbc = dt("negbc", [P, NCH, 4], F32, kind="ExternalInput").ap()
    bdic = dt("bdic", [P, NCH, 4], F32, kind="ExternalInput").ap()
    biasAc = dt("biasAc", [P, NCH, 4], F32, kind="ExternalInput").ap()
    negcc = dt("negcc", [P, NCH, 4], F32, kind="ExternalInput").ap()
    dratc = dt("dratc", [P, NCH, 4], F32, kind="ExternalInput").ap()
    gendc = dt("gendc", [P, NCH, 4], F32, kind="ExternalInput").ap()
    mamT = dt("mamT", [P, P], BF16, kind="ExternalInput").ap()
    mpmT = dt("mpmT", [P, P], BF16, kind="ExternalInput").ap()
    idb = dt("idb", [P, P], BF16, kind="ExternalInput").ap()
    idf = dt("idf", [P, P], F32, kind="ExternalInput").ap()
    p2 = dt("p2", [T, D], BF16, kind="ExternalOutput").ap()

    with tile.TileContext(nc) as tc:
      with tc.tile_pool(name="res", bufs=1) as res:
        # whole-kernel residents
        qkv_sb = res.tile([P, 8, T], BF16)      # conv+silu outputs [f, t]
        zs_sb = res.tile([P, TT, 512], BF16)    # silu(z) [t, f] (nw in wout)
        s2bc_sb = res.tile([P, T], BF16)        # scale2 bcast rows [f, t]
        scale2_sb = res.tile([P, TT], F32)
        S_all = res.tile([P, NCH + 1, 4, DV], BF16)  # chunk-entry states
        cum_sb = res.tile([P, NCH, 4], F32)
        beta_sb = res.tile([P, NCH, 4], F32)
        negb_sb = res.tile([P, NCH, 4], F32)
        bdi_sb = res.tile([P, NCH, 4], F32)
        biasA_sb = res.tile([P, NCH, 4], F32)
        negc_sb = res.tile([P, NCH, 4], F32)
        drat_sb = res.tile([P, NCH, 4], F32)
        gend_sb = res.tile([P, NCH, 4], F32)
        cw_sb = res.tile([P, 8, KCONV], F32)
        mamT_sb = res.tile([P, P], BF16)
        mpmT_sb = res.tile([P, P], BF16)
        idb_sb = res.tile([P, P], BF16)
        idf_sb = res.tile([P, P], F32)
        ones_f = res.tile([P, P], F32)
        onescol_sb = res.tile([P, 1], BF16)
        wout_sb = res.tile([P, 4, D], BF16)
        epsc = res.tile([P, 1], F32)
        lnepsc = res.tile([P, 1], F32)
        nc.vector.memset(S_all[:, 0, :, :], 0.0)
        nc.vector.memset(ones_f[:], 1.0)
        nc.vector.memset(onescol_sb[:], 1.0)
        nc.vector.memset(epsc[:], EPS)
        nc.vector.memset(lnepsc[:], LNEPS)
        nc.sync.dma_start(cw_sb[:], cwt[:])
        nc.sync.dma_start(s2bc_sb[:], s2bc[:])
        nc.sync.dma_start(scale2_sb[:], s2c[:])
        nc.sync.dma_start(cum_sb[:], cumc[:])
        nc.sync.dma_start(beta_sb[:], betac[:])
        nc.sync.dma_start(negb_sb[:], negbc[:])
        nc.sync.dma_start(bdi_sb[:], bdic[:])
        nc.sync.dma_start(biasA_sb[:], biasAc[:])
        nc.sync.dma_start(negc_sb[:], negcc[:])
        nc.sync.dma_start(drat_sb[:], dratc[:])
        nc.sync.dma_start(gend_sb[:], gendc[:])
        nc.sync.dma_start(mamT_sb[:], mamT[:])
        nc.sync.dma_start(mpmT_sb[:], mpmT[:])
        nc.sync.dma_start(idb_sb[:], idb[:])
        nc.sync.dma_start(idf_sb[:], idf[:])
        nc.sync.dma_start(
            wout_sb[:],
            bass.AP(wout.tensor, 0, [[D, P], [P * D, 4], [1, D]]))

        # ============ phase B/C/D: projections + conv (silu table) =========
        with tc.tile_pool(name="big1", bufs=1) as big1, \
             tc.tile_pool(name="hTp", bufs=2) as hTp, \
             tc.tile_pool(name="mxp", bufs=10) as mxp, \
             tc.tile_pool(name="wk1", bufs=4) as wk1, \
             tc.tile_pool(name="psB", bufs=2, space="PSUM") as psB, \
             tc.tile_pool(name="psZ", bufs=2, space="PSUM") as psZ:
            wqkv_sb = big1.tile([P, KT, 1024], BF16)
            wz_sb = big1.tile([P, KT, 512], BF16)
            nc.sync.dma_start(
                wqkv_sb[:],
                bass.AP(wqkv.tensor, 0, [[1024, P], [P * 1024, KT],
                                         [1, 1024]]))
            nc.sync.dma_start(
                wz_sb[:],
                bass.AP(wz.tensor, 0, [[512, P], [P * 512, KT], [1, 512]]))

            prev_mx = [None] * 8
            for n4 in range(4):
                hT_n = hTp.tile([P, KT, 512], BF16, tag="hTn")
                nc.sync.dma_start(
                    hT_n[:],
                    bass.AP(hT.tensor, n4 * 512, [[T, P], [P * T, KT],
                                                  [1, 512]]))
                for F in range(8):
                    pm = psB.tile([P, 512], F32, tag="pm")
                    for k in range(KT):
                        nc.tensor.matmul(pm[:], wqkv_sb[:, k, ts(F, P)],
                                         hT_n[:, k, :],
                                         start=(k == 0), stop=(k == KT - 1))
                    m1 = mxp.tile([P, 515], BF16, tag="mxc")
                    nc.vector.tensor_tensor(m1[:, 3:515], pm[:],
                                            s2bc_sb[:, ts(n4, 512)], ALU.mult)
                    if n4 == 0:
                        nc.vector.memset(m1[:, 0:3], 0.0)
                    else:
                        nc.vector.tensor_copy(m1[:, 0:3],
                                              prev_mx[F][:, 512:515])
                    prev_mx[F] = m1
                    c0 = wk1.tile([P, 512], BF16, tag="cc0")
                    nc.vector.tensor_scalar(c0[:], m1[:, 0:512],
                                            cw_sb[:, F, 0:1], None, ALU.mult)
                    for j in range(1, KCONV):
                        c1 = wk1.tile([P, 512], BF16, tag=f"cc{j % 2}")
                        nc.vector.scalar_tensor_tensor(
                            c1[:], m1[:, j:512 + j], cw_sb[:, F, j:j + 1],
                            c0[:], ALU.mult, ALU.add)
                        c0 = c1
                    nc.scalar.activation(qkv_sb[:, F, ts(n4, 512)], c0[:],
                                         AF.Silu)
                # z for the 4 token tiles in this superblock
                for m in range(4 * n4, 4 * n4 + 4):
                    pz = psZ.tile([P, 512], F32, tag="pz")
                    for k in range(KT):
                        nc.tensor.matmul(pz[:], hT_n[:, k, ts(m - 4 * n4, P)],
                                         wz_sb[:, k, :],
                                         start=(k == 0), stop=(k == KT - 1))
                    zraw = wk1.tile([P, 512], F32, tag="zraw")
                    nc.vector.tensor_scalar(zraw[:], pz[:],
                                            scale2_sb[:, m:m + 1], None,
                                            ALU.mult)
                    nc.scalar.activation(zs_sb[:, m, :], zraw[:], AF.Silu)

        tc.strict_bb_all_engine_barrier()
        _load_act_set(nc, 6)  # natural_log_exp_and_others
        # ============ phase E: per-chunk operators + scan + out (exp/ln) ===
        # PSUM slots (8 banks): pb x2, psA x2, pgx x1, pk x1, pso x1, pout x1
        with tc.tile_pool(name="wkE", bufs=10) as wkE, \
             tc.tile_pool(name="wkE2", bufs=6) as wkE2, \
             tc.tile_pool(name="wkEs", bufs=24) as wkEs, \
             tc.tile_pool(name="sqs", bufs=4) as sqs, \
             tc.tile_pool(name="ogp", bufs=10) as ogp, \
             tc.tile_pool(name="psE", bufs=1, space="PSUM") as psE:
            for n in range(NCH):
                cum4 = cum_sb[:, n, :]
                beta4 = beta_sb[:, n, :]
                negb4 = negb_sb[:, n, :]
                bdi4 = bdi_sb[:, n, :]
                biasA4 = biasA_sb[:, n, :]
                negc4 = negc_sb[:, n, :]
                drat4 = drat_sb[:, n, :]
                gend4 = gend_sb[:, n, :]

                # ---- per-kh: k transpose (DMA), norms, grams, rq ----
                ktms = []; rkcs = []; lnsqs = []
                gram_sbs = []; lnrqs = []; rq2s = []
                for kh in range(2):
                    pgx = psE.tile([P, 392], F32, tag="pgx")
                    kfm = qkv_sb[:, 2 + kh, ts(n, P)]
                    qfm = qkv_sb[:, kh, ts(n, P)]
                    ktm = wkE.tile([P, P], BF16, tag="ktm")
                    nc.sync.dma_start(ktm[:], kfm, transpose=True)
                    ktms.append(ktm)
                    sqk = sqs.tile([P, P], BF16, tag="sqk")
                    ssqk = wkEs.tile([P, 1], F32, tag="ssqk")
                    nc.vector.scalar_tensor_tensor(sqk[:], ktm[:], 1.0,
                                                   ktm[:], ALU.mult, ALU.mult,
                                                   accum_out=ssqk[:])
                    lnsq = wkEs.tile([P, 1], F32, tag="lnsq")
                    nc.scalar.activation(lnsq[:], ssqk[:], AF.Ln,
                                         bias=lnepsc[:])
                    lnsqs.append(lnsq)
                    rkc = wkEs.tile([P, 1], F32, tag="rkc")
                    nc.scalar.activation(rkc[:], lnsq[:], AF.Exp, scale=-0.5)
                    rkcs.append(rkc)
                    lnrk = wkEs.tile([P, 1], F32, tag="lnrk")
                    nc.vector.tensor_scalar_mul(lnrk[:], lnsq[:], -0.5)
                    lnrkbc = wkE.tile([P, P], F32, tag="lnrkbc")
                    nc.vector.tensor_scalar(lnrkbc[:], ones_f[:], lnrk[:],
                                            None, ALU.mult)
                    # raw grams (rk folded into exp bias / accum)
                    nc.tensor.matmul(pgx[:, 0:128], kfm, kfm,
                                     start=True, stop=True)
                    nc.tensor.matmul(pgx[:, 128:256], kfm, qfm,
                                     start=True, stop=True)
                    gram_sb = wkE.tile([P, 256], BF16, tag="gram")
                    nc.scalar.activation(gram_sb[:], pgx[:, 0:256], AF.Copy)
                    gram_sbs.append(gram_sb)
                    # rq: colsum of q^2 -> ln -> transpose -> cols
                    sqq = sqs.tile([P, P], BF16, tag="sqq")
                    nc.vector.tensor_tensor(sqq[:], qfm, qfm, ALU.mult)
                    pq = pgx[0:1, 264:392]
                    nc.tensor.matmul(pq, onescol_sb[:], sqq[:],
                                     start=True, stop=True)
                    lnrow = wkEs.tile([1, P], F32, tag="lnrow")
                    nc.scalar.activation(lnrow[:], pq, AF.Ln,
                                         scale=float(DK),
                                         bias=lnepsc[0:1, :])
                    ptq = pgx[:, 260:261]
                    nc.tensor.transpose(ptq, lnrow[:], idf_sb[0:1, 0:1])
                    lnrq = wkEs.tile([P, 1], F32, tag="lnrq")
                    nc.scalar.activation(lnrq[:], ptq, AF.Copy, scale=-0.5)
                    lnrqs.append(lnrq)
                    rq2 = wkEs.tile([P, 1], F32, tag="rq2")
                    nc.scalar.activation(rq2[:], lnrq[:], AF.Exp, scale=2.0)
                    rq2s.append(rq2)

                # ---- per-head operator build ----
                ssqo4 = wkEs.tile([P, 4], F32, tag="ssqo4")
                o_pss = []
                Meffs = []; Kuvs = []; qeffs = []; ovs = []
                for h in range(4):
                    kh = h // 2
                    qfm = qkv_sb[:, kh, ts(n, P)]
                    vfm = qkv_sb[:, 4 + h, ts(n, P)]
                    ktm = ktms[kh]; gram_sb = gram_sbs[kh]
                    cbc = wkE2.tile([P, P], F32, tag="cbc")
                    nc.vector.tensor_scalar(cbc[:], ones_f[:],
                                            cum4[:, h:h + 1], None, ALU.mult)
                    pb3 = psE.tile([P, 384], F32, tag="pb", bufs=2)
                    pbam = pb3[:, 0:128]
                    pbpm = pb3[:, 128:256]
                    pbdi = pb3[:, 256:384]
                    nc.tensor.matmul(pbam, cbc[:], idf_sb[:],
                                     is_transpose=True, start=True,
                                     stop=False)
                    nc.tensor.matmul(pbam, lnrkbc[:], idf_sb[:],
                                     is_transpose=True, start=False,
                                     stop=False)
                    nc.tensor.matmul(pbam, idb_sb[:], mamT_sb[:],
                                     start=False, stop=True)
                    nc.tensor.matmul(pbpm, cbc[:], idf_sb[:],
                                     is_transpose=True, start=True,
                                     stop=False)
                    nc.tensor.matmul(pbpm, idb_sb[:], mpmT_sb[:],
                                     start=False, stop=True)
                    nc.tensor.transpose(pbdi, cbc[:], idf_sb[:])
                    # biases: A: lnb - cum + lnrk ; P: -cum + lnrk
                    bA = wkEs.tile([P, 1], F32, tag="bA")
                    nc.vector.scalar_tensor_tensor(bA[:], lnsqs[kh][:], -0.5,
                                                   biasA4[:, h:h + 1],
                                                   ALU.mult, ALU.add)
                    bP = wkEs.tile([P, 1], F32, tag="bP")
                    nc.vector.scalar_tensor_tensor(bP[:], lnsqs[kh][:], -0.5,
                                                   negc4[:, h:h + 1],
                                                   ALU.mult, ALU.add)
                    amt = wkE2.tile([P, P], BF16, tag="amt")
                    nc.scalar.activation(amt[:], pbam, AF.Exp, bias=bA[:])
                    pmt = wkE2.tile([P, P], BF16, tag="pmt")
                    nc.scalar.activation(pmt[:], pbpm, AF.Exp, bias=bP[:])
                    dib = wkE2.tile([P, P], BF16, tag="dib")
                    nc.scalar.activation(dib[:], pbdi, AF.Exp)
                    abar = wkE2.tile([P, P], BF16, tag="abar")
                    nc.vector.tensor_tensor(abar[:], amt[:],
                                            gram_sb[:, 0:128], ALU.mult)
                    pbar = wkE2.tile([P, P], BF16, tag="pbar")
                    nc.vector.tensor_tensor(pbar[:], pmt[:],
                                            gram_sb[:, 128:256], ALU.mult)
                    # RHS R = [beta*v | beta*di*rk*k] (token-major)
                    R = wkE2.tile([P, 256], BF16, tag="R")
                    vtm = ogp.tile([P, P], BF16, tag="vtm")
                    nc.sync.dma_start(vtm[:], vfm, transpose=True)
                    nc.scalar.activation(R[:, 0:128], vtm[:], AF.Copy,
                                         scale=beta4[:, h:h + 1])
                    bdirk = wkEs.tile([P, 1], F32, tag="bdirk")
                    nc.vector.tensor_tensor(bdirk[:], bdi4[:, h:h + 1],
                                            rkcs[kh][:], ALU.mult)
                    nc.vector.tensor_scalar(R[:, 128:256], ktm[:], bdirk[:],
                                            None, ALU.mult)
                    # u = R - beta*(Atil @ R)  (Neumann order 1)
                    psA = psE.tile([P, 256], F32, tag="psA", bufs=2)
                    nc.tensor.matmul(psA[:], abar[:], R[:], start=True,
                                     stop=True)
                    u = wkE2.tile([P, 256], BF16, tag="u")
                    nc.vector.scalar_tensor_tensor(u[:], psA[:],
                                                   negb4[:, h:h + 1], R[:],
                                                   ALU.mult, ALU.add)
                    uv = u[:, 0:128]; G = u[:, 128:256]
                    # pk regions: PG | ov | Kuv | KG
                    pk4 = psE.tile([P, 512], F32, tag="pk")
                    pPG = pk4[:, 0:128]
                    pov = pk4[:, 128:256]
                    pKuv = pk4[:, 256:384]
                    pKG = pk4[:, 384:512]
                    nc.tensor.matmul(pPG, G, pbar[:], start=True, stop=True)
                    qtil = wkE2.tile([P, P], BF16, tag="qtil")
                    nc.vector.tensor_tensor(qtil[:], qfm, dib[:], ALU.mult)
                    qeffT = wkE.tile([P, P], BF16, tag="qeffT")
                    nc.vector.tensor_tensor(qeffT[:], qtil[:], pPG,
                                            ALU.subtract)
                    qeffs.append(qeffT)
                    rkdr = wkEs.tile([P, 1], F32, tag="rkdr")
                    nc.vector.tensor_tensor(rkdr[:], drat4[:, h:h + 1],
                                            rkcs[kh][:], ALU.mult)
                    ktil = wkE2.tile([P, P], BF16, tag="ktil")
                    nc.vector.tensor_scalar(ktil[:], ktm[:], rkdr[:], None,
                                            ALU.mult)
                    nc.tensor.matmul(pov, pbar[:], uv, start=True, stop=True)
                    ov = wkE.tile([P, P], BF16, tag="ov")
                    nc.vector.tensor_copy(ov[:], pov)
                    ovs.append(ov)
                    nc.tensor.matmul(pKuv, ktil[:], uv, start=True, stop=True)
                    Kuv = wkE.tile([P, P], BF16, tag="Kuv")
                    nc.scalar.activation(Kuv[:], pKuv, AF.Copy)
                    Kuvs.append(Kuv)
                    nc.tensor.matmul(pKG, G, ktil[:], start=True, stop=True)
                    MeffT = wkE.tile([P, P], BF16, tag="MeffT")
                    nc.vector.scalar_tensor_tensor(MeffT[:], idf_sb[:],
                                                   gend4[:, h:h + 1], pKG,
                                                   ALU.mult, ALU.subtract)
                    Meffs.append(MeffT)

                # ---- scan + o-recovery + gating for this chunk ----
                for h in range(4):
                    pso = psE.tile([P, 256], F32, tag="pso")
                    psc = pso[:, 0:128]
                    po = pso[:, 128:256]
                    nc.tensor.matmul(psc, idb_sb[:], Kuvs[h][:],
                                     start=True, stop=False)
                    nc.tensor.matmul(psc, Meffs[h][:], S_all[:, n, h, :],
                                     start=False, stop=True)
                    nc.scalar.activation(S_all[:, n + 1, h, :], psc,
                                         AF.Copy)
                    nc.tensor.matmul(po, idb_sb[:], ovs[h][:],
                                     start=True, stop=False)
                    nc.tensor.matmul(po, qeffs[h][:], S_all[:, n, h, :],
                                     start=False, stop=True)
                    o_sb = ogp.tile([P, P], BF16, tag="o_sb")
                    nc.scalar.activation(o_sb[:], po, AF.Copy)
                    o_pss.append(o_sb)
                    sqo = sqs.tile([P, P], BF16, tag="sqo")
                    nc.vector.scalar_tensor_tensor(
                        sqo[:], o_sb[:], 1.0, o_sb[:], ALU.mult, ALU.mult,
                        accum_out=ssqo4[:, h:h + 1])
                t4 = wkEs.tile([P, 4], F32, tag="t4")
                nc.vector.tensor_scalar(t4[:, 0:2], ssqo4[:, 0:2],
                                        rq2s[0][:], None, ALU.mult)
                nc.vector.tensor_scalar(t4[:, 2:4], ssqo4[:, 2:4],
                                        rq2s[1][:], None, ALU.mult)
                lnt4 = wkEs.tile([P, 4], F32, tag="lnt4")
                nc.scalar.activation(lnt4[:], t4[:], AF.Ln,
                                     scale=1.0 / DV, bias=epsc[:])
                ogTs = []
                for h in range(4):
                    fh = wkEs.tile([P, 1], F32, tag="fh")
                    nc.scalar.activation(fh[:], lnt4[:, h:h + 1], AF.Exp,
                                         scale=-0.5,
                                         bias=lnrqs[h // 2][:])
                    og = ogp.tile([P, P], BF16, tag="og")
                    nc.vector.scalar_tensor_tensor(og[:], o_pss[h][:],
                                                   fh[:],
                                                   zs_sb[:, n, ts(h, P)],
                                                   ALU.mult, ALU.mult)
                    ogT = ogp.tile([P, P], BF16, tag="ogT")
                    nc.sync.dma_start(ogT[:], og[:], transpose=True)
                    ogTs.append(ogT)
                # ---- out projection for this chunk (single p2 store) ----
                ot = ogp.tile([P, 4, 512], BF16, tag="ot", bufs=2)
                for nn in range(4):
                    pout = psE.tile([P, 512], F32, tag="pout")
                    for h in range(4):
                        nc.tensor.matmul(pout[:], ogTs[h][:],
                                         wout_sb[:, h, ts(nn, 512)],
                                         start=(h == 0), stop=(h == 3))
                    if nn % 2 == 0:
                        nc.scalar.activation(ot[:, nn, :], pout[:], AF.Copy)
                    else:
                        nc.vector.tensor_copy(ot[:, nn, :], pout[:])
                nc.sync.dma_start(p2[ts(n, P), :], ot[:])
    nc.compile()
    return nc


def _prep_delta_inputs(h, ln2_w, dn_qkv_w, dn_z_w, dn_b_w, dn_a_w, conv_w,
                       dt_bias, A_log, dn_norm_w, dn_out_w):
    ln2f = (1.0 + ln2_w.astype(np.float32))
    h32 = h.astype(np.float32)
    ssq = np.mean(h32 * h32, axis=1)                       # [T]
    s2 = 1.0 / np.sqrt(ssq + EPS)                          # [T]
    s2c_np = np.ascontiguousarray(s2.reshape(TT, P).T).astype(np.float32)
    s2bc_np = _bf(np.tile(s2[None, :], (P, 1)))
    hT_np = _bf(h.T)
    # host gate columns (per core below: slices of these [T, HV] arrays)
    hn = h32 * s2[:, None]                                 # normed (pre-ln2f)
    aa = (hn * ln2f[None, :]) @ dn_a_w.T                   # [T, HV]
    bb = (hn * ln2f[None, :]) @ dn_b_w.T
    beta = 1.0 / (1.0 + np.exp(-bb))
    lnb = -np.log1p(np.exp(-bb))
    g = -np.exp(A_log)[None, :] * np.log1p(np.exp(aa + dt_bias[None, :]))
    cum = np.cumsum(g.reshape(NCH, P, HV), axis=1)         # [NCH, P, HV]
    di = np.exp(cum)
    gend = np.exp(cum[:, -1:, :])                          # [NCH, 1, HV]
    drat = np.exp(cum[:, -1:, :] - cum)
    betac_ = beta.reshape(NCH, P, HV)
    lnbc_ = lnb.reshape(NCH, P, HV)
    a2 = np.arange(P)
    mamT_np = _bf(np.where(a2[None, :] > a2[:, None], 0.0, MASKV))
    mpmT_np = _bf(np.where(a2[None, :] >= a2[:, None], 0.0, MASKV))
    idb_np = _bf(np.eye(P, dtype=np.float32))
    idf_np = np.eye(P, dtype=np.float32)
    nwf = dn_norm_w.astype(np.float32)

    def cols(arr, vhs):  # [NCH, P, HV] -> [P, NCH, 4]
        return np.ascontiguousarray(
            arr[:, :, vhs].transpose(1, 0, 2)).astype(np.float32)

    ins = []
    for c in range(NCORE):
        khs = [2 * c, 2 * c + 1]
        vhs = [4 * c + j for j in range(4)]
        qrows = np.concatenate([dn_qkv_w[kh * DK:(kh + 1) * DK] for kh in khs])
        krows = np.concatenate([dn_qkv_w[KEY_DIM + kh * DK:
                                         KEY_DIM + (kh + 1) * DK] for kh in khs])
        vrows = dn_qkv_w[2 * KEY_DIM + vhs[0] * DV:
                         2 * KEY_DIM + (vhs[-1] + 1) * DV]
        rows = np.concatenate([qrows, krows, vrows])  # [1024, D]
        wqkv_np = _bf((rows * ln2f[None, :]).T)
        crow_q = np.concatenate([conv_w[kh * DK:(kh + 1) * DK, 0, :]
                                 for kh in khs])
        crow_k = np.concatenate([conv_w[KEY_DIM + kh * DK:
                                        KEY_DIM + (kh + 1) * DK, 0, :]
                                 for kh in khs])
        crow_v = conv_w[2 * KEY_DIM + vhs[0] * DV:
                        2 * KEY_DIM + (vhs[-1] + 1) * DV, 0, :]
        crows = np.concatenate([crow_q, crow_k, crow_v])  # [1024, 4]
        cwt_np = np.ascontiguousarray(
            crows.reshape(8, P, KCONV).transpose(1, 0, 2).reshape(P, 8 * KCONV)
        ).astype(np.float32)
        zrows = dn_z_w[vhs[0] * DV:(vhs[-1] + 1) * DV]
        wz_np = _bf((zrows * ln2f[None, :]).T)
        wout_np = _bf(dn_out_w[:, vhs[0] * DV:(vhs[-1] + 1) * DV].T
                      * np.tile(nwf, 4)[:, None])
        ins.append(dict(
            hT=hT_np, wqkv=wqkv_np, cwt=cwt_np, wz=wz_np, wout=wout_np,
            s2c=s2c_np, s2bc=s2bc_np,
            cumc=cols(cum, vhs),
            betac=cols(betac_, vhs),
            negbc=cols(-betac_, vhs),
            bdic=cols(betac_ * di, vhs),
            biasAc=cols(lnbc_ - cum, vhs),
            negcc=cols(-cum, vhs),
            dratc=cols(drat * np.ones_like(cum), vhs),
            gendc=cols(gend * np.ones_like(cum), vhs),
            mamT=mamT_np, mpmT=mpmT_np, idb=idb_np, idf=idf_np))
    return ins


def _get_delta_nc():
    if "delta" not in _CACHE:
        _CACHE["delta"] = build_delta()
    return _CACHE["delta"]


def run_delta(h, ln2_w, dn_qkv_w, dn_z_w, dn_b_w, dn_a_w, conv_w,
              dt_bias, A_log, dn_norm_w, dn_out_w):
    nc2 = _get_delta_nc()
    ins2 = _prep_delta_inputs(h, ln2_w, dn_qkv_w, dn_z_w, dn_b_w, dn_a_w,
                              conv_w, dt_bias, A_log, dn_norm_w, dn_out_w)
    res2 = run_bass_kernel_spmd(nc2, ins2, core_ids=list(range(NCORE)))
    out = h.astype(np.float32).copy()
    for c in range(NCORE):
        out += res2.results[c]["p2"].astype(np.float32)
    return out


# ============================================================ host helpers
def _bf(a):
    return np.ascontiguousarray(a.astype(BFNP))


def _prep_attn_inputs(x, input_pos, ln1_w, q_w, k_w, v_w, o_w, qn_w, kn_w):
    x2 = x.reshape(T, D).astype(np.float32)
    ln1f = (1.0 + ln1_w.astype(np.float32))
    xT_np = _bf(x2.T)
    xr_np = _bf(x2)
    inv_freq = 1.0 / THETA ** (np.arange(0, ROT, 2, dtype=np.float32) / ROT)
    fr = input_pos.astype(np.float32)[:, None] * inv_freq[None, :]
    cos = np.cos(fr).astype(np.float32); sin = np.sin(fr).astype(np.float32)
    csd_np = np.concatenate([cos, cos, sin, sin], axis=1)
    qk1_np = _bf(np.concatenate(
        [np.tile(1.0 + qn_w[None, :], (P, 1)),
         np.tile(1.0 + kn_w[None, :], (P, 1))], axis=1))
    a = np.arange(P)[:, None]; b = np.arange(512)[None, :]
    m4_np = _bf(np.concatenate(
        [(a + 128 * r <= b).astype(np.float32) for r in range(4)], axis=1))
    idm_np = _bf(np.eye(P, dtype=np.float32))
    ins = []
    for c in range(NCORE):
        qh = [2 * c, 2 * c + 1]; kvh = c // 2
        qrows = np.concatenate([q_w[h * 256: h * 256 + 128] for h in qh]
                               + [q_w[h * 256 + 128: h * 256 + 256] for h in qh])
        wqg_np = _bf((qrows * ln1f[None, :]).T)
        kvrows = np.concatenate([k_w[kvh * 128: kvh * 128 + 128],
                                 v_w[kvh * 128: kvh * 128 + 128]])
        wkv_np = _bf((kvrows * ln1f[None, :]).T)
        wo_np = _bf(o_w[:, 2 * c * 128: 2 * c * 128 + 256].T)
        ins.append(dict(xT=xT_np, xr=xr_np, wqg=wqg_np, wkv=wkv_np,
                        wo=wo_np, csd=csd_np, qk1=qk1_np, m4=m4_np,
                        idm=idm_np))
    return ins


_CACHE = {}


def _get_attn_nc():
    if "attn" not in _CACHE:
        _CACHE["attn"] = build_attn()
    return _CACHE["attn"]


def kernel(x, input_pos, ln1_w, ln2_w, q_w, k_w, v_w, o_w, qn_w, kn_w,
           dn_qkv_w, dn_z_w, dn_b_w, dn_a_w, conv_w, dt_bias, A_log,
           dn_norm_w, dn_out_w):
    x = np.asarray(x); input_pos = np.asarray(input_pos)
    args = dict(x=x, input_pos=input_pos, ln1_w=np.asarray(ln1_w),
                ln2_w=np.asarray(ln2_w), q_w=np.asarray(q_w),
                k_w=np.asarray(k_w), v_w=np.asarray(v_w), o_w=np.asarray(o_w),
                qn_w=np.asarray(qn_w), kn_w=np.asarray(kn_w))
    nc1 = _get_attn_nc()
    ins1 = _prep_attn_inputs(x, input_pos, args["ln1_w"], args["q_w"],
                             args["k_w"], args["v_w"], args["o_w"],
                             args["qn_w"], args["kn_w"])
    res1 = run_bass_kernel_spmd(nc1, ins1, core_ids=list(range(NCORE)))
    h = x.reshape(T, D).astype(np.float32).copy()
    for c in range(NCORE):
        h += res1.results[c]["p1"]

    out = run_delta(h, np.asarray(ln2_w), np.asarray(dn_qkv_w),
                    np.asarray(dn_z_w), np.asarray(dn_b_w),
                    np.asarray(dn_a_w), np.asarray(conv_w),
                    np.asarray(dt_bias), np.asarray(A_log),
                    np.asarray(dn_norm_w), np.asarray(dn_out_w))
    return out.reshape(B, T, D).astype(np.float32)

